# revision 6
# baseline (speedup 1.0000x reference)
"""GCNConv (add self-loops, symmetric norm, linear, relu, broadcast) on 8 TRN2 cores.

Cores share HBM in pairs (0,1), (2,3), (4,5), (6,7).  Each pair cooperatively
computes the full h = x @ W into a pair-shared DRAM tensor: the even core
computes rows [0, 5120), the odd core rows [5120, 10240) (node id == h row).
Destination nodes are row-sharded across the 8 cores (1250 rows each).  For
each 128-row destination tile the core gathers the deduplicated source-node h
rows with dma_gather and reduces them with PE matmuls against host-built block
scatter matrices S (S[u, d] = summed edge norm from source-slot u into local
destination d).  Source slots are grouped [a1 | a2 | b]: a1/a2 are the two
quarters of the half this core computes itself (gathers may start as soon as
the local phase-0 writes land), b is the partner's half (gathers wait on a
tiny pair-AllGather barrier triggered from the scalar engine).  Bias-add +
relu on DVE; the (N,H,E,R) broadcast expansion happens on host at unshard.
"""

import numpy as np
import ml_dtypes

import concourse.bacc as bacc
import concourse.bass as bass
import concourse.mybir as mybir
import concourse.tile as tile
from concourse.bass_utils import run_bass_kernel_spmd

N_NODES = 10000
N_GENES = 978
EMBED = 301
HEADS = 4
REP = 12
N_CORES = 8
NPC = N_NODES // N_CORES          # 1250 dst rows per core
DT = 128                          # dst tile height
NT = (NPC + DT - 1) // DT         # 10 dst tiles per core
GP = 1024                         # padded gene dim (8 chunks of 128)
SP = 10240                        # padded node dim for h
HALF = SP // 2                    # rows computed per core (pair covers SP)
QUART = HALF // 2
PAIRS = [[0, 1], [2, 3], [4, 5], [6, 7]]
# h row padding: gather elem_size_bytes must be a multiple of 256
# -> 384 f16 elems (768 B)

F32 = mybir.dt.float32
BF16 = mybir.dt.bfloat16
F16 = mybir.dt.float16
I16 = mybir.dt.int16

VARIANTS = {
    "f32": dict(x_dt=F32, h_dt=F32, mm1_dt=F32, out_dt=F32),
    "f16": dict(x_dt=F16, h_dt=F16, mm1_dt=F16, out_dt=F32),
    "f16o": dict(x_dt=F16, h_dt=F16, mm1_dt=F16, out_dt=F16),
}
VARIANT = "f16o"

_prog_cache: dict = {}


def _np_dt(dt):
    return {F32: np.float32, BF16: ml_dtypes.bfloat16, F16: np.float16}[dt]


def _build_program(bmaxes: tuple, variant: str):
    cfg = VARIANTS[variant]
    x_dt, h_dt, mm1_dt, out_dt = (cfg["x_dt"], cfg["h_dt"], cfg["mm1_dt"],
                                  cfg["out_dt"])
    HROW = 320 if h_dt == F32 else 384
    b1, b2, bb = bmaxes
    bmax = b1 + b2 + bb
    slots = bmax * 128
    GCH = GP // 128
    nc = bacc.Bacc("TRN2", target_bir_lowering=False, debug=False,
                   num_devices=N_CORES, num_swdge_queues=4)

    xT_d = nc.dram_tensor("xT", [GP, HALF], x_dt, kind="ExternalInput")
    W_d = nc.dram_tensor("Wp", [GP, EMBED], x_dt, kind="ExternalInput")
    b_d = nc.dram_tensor("bB", [128, EMBED], F32, kind="ExternalInput")
    S_d = nc.dram_tensor("Sblk", [NT, 128, slots], mm1_dt, kind="ExternalInput")
    ix_d = nc.dram_tensor("idxw", [NT, 128, slots // 16], I16, kind="ExternalInput")
    out_d = nc.dram_tensor("out", [NPC, EMBED], out_dt, kind="ExternalOutput")
    h_d = nc.dram_tensor("hbuf", [SP, HROW], h_dt, addr_space="Shared")
    bar_in = nc.dram_tensor("barin", [1, 8], mybir.dt.uint8)
    bar_out = nc.dram_tensor("barout", [2, 8], mybir.dt.uint8)

    with tile.TileContext(nc) as tc:
        with (
            tc.tile_pool(name="const", bufs=1) as cpool,
            tc.tile_pool(name="sS", bufs=5) as spool,
            tc.tile_pool(name="sI", bufs=5) as ipool,
        ):
            b_sb = cpool.tile([128, EMBED], F32)
            nc.sync.dma_start(b_sb[:], b_d[:])

            # prefetch the first tiles' S/ix during phase 0
            pre = {}
            for t in range(4):
                s_sb = spool.tile([128, slots], mm1_dt, tag="s")
                nc.sync.dma_start(s_sb[:], S_d[t])
                ix_sb = ipool.tile([128, slots // 16], I16, tag="ix")
                nc.sync.dma_start(ix_sb[:], ix_d[t])
                pre[t] = (s_sb, ix_sb)

            # ---------------- phase 0: h(my half) = x @ W ----------------
            # parity 0 computes rows [0, HALF), parity 1 rows [HALF, SP)
            pid = nc.scalar.partition_id()
            parity = pid - (pid // 2) * 2
            hbase = parity * (HALF * HROW)

            sents = {}
            all_h_writes = []
            with (
                tc.tile_pool(name="wsb", bufs=1) as wpool,
                tc.tile_pool(name="xt", bufs=24) as xpool,
                tc.tile_pool(name="hsb", bufs=4) as hpool,
                tc.tile_pool(name="ph", bufs=6, space="PSUM") as phpool,
            ):
                w_sb = wpool.tile([128, GCH, EMBED], x_dt)
                for g in range(GCH):
                    nc.sync.dma_start(w_sb[:, g, :], W_d[g * 128:(g + 1) * 128, :])

                SG = 512
                for lo, hi, key in ((0, QUART, "a1"), (QUART, HALF, "a2")):
                    h_writes = []
                    for s0 in range(lo, hi, SG):
                        sgw = min(SG, hi - s0)
                        xts = []
                        for g in range(GCH):
                            xt = xpool.tile([128, SG], x_dt, tag="xt")
                            nc.sync.dma_start(
                                xt[:, :sgw],
                                xT_d[g * 128:(g + 1) * 128, s0:s0 + sgw])
                            xts.append(xt)
                        for sub in range(sgw // 128):
                            ph = phpool.tile([128, EMBED], F32)
                            for g in range(GCH):
                                nc.tensor.matmul(
                                    ph[:],
                                    xts[g][:, sub * 128:(sub + 1) * 128],
                                    w_sb[:, g, :],
                                    start=(g == 0), stop=(g == GCH - 1),
                                )
                            h_sb = hpool.tile([128, EMBED], h_dt)
                            nc.vector.tensor_copy(h_sb[:], ph[:])
                            dst = bass.AP(
                                h_d,
                                hbase + (s0 + sub * 128) * HROW,
                                [[HROW, 128], [1, EMBED]])
                            h_writes.append(nc.scalar.dma_start(dst, h_sb[:]))
                    sent = nc.sync.nop()
                    for hw in h_writes:
                        tile.add_dep_helper(sent.ins, hw.ins,
                                            reason=f"h-{key} ready")
                    sents[key] = sent
                    all_h_writes += h_writes

            # pair barrier: partner half visible after this completes
            cc = nc.gpsimd.collective_compute(
                "AllGather", mybir.AluOpType.bypass,
                replica_groups=PAIRS,
                ins=[bar_in[:]], outs=[bar_out[:]],
            )
            for hw in all_h_writes:
                tile.add_dep_helper(cc.ins, hw.ins, reason="barrier waits h")
            sents["b"] = cc

            # ------------- phase 1: aggregate + bias + relu -------
            with (
                tc.tile_pool(name="sG", bufs=14) as gpool,
                tc.tile_pool(name="sO", bufs=3) as opool,
                tc.tile_pool(name="pO", bufs=8, space="PSUM") as popool,
            ):
                GBLK = 8
                qctr = {"a": [0], "b": [0]}

                def gather_group(ix_sb, blk0, blk1, sent, qcls, qbase):
                    out = []
                    for b0 in range(blk0, blk1, GBLK):
                        nb = min(GBLK, blk1 - b0)
                        g_sb = gpool.tile([128, GBLK, HROW], h_dt, tag="g")
                        gi = nc.gpsimd.dma_gather(
                            g_sb[:, :nb, :], h_d[:],
                            ix_sb[:, b0 * 8:(b0 + nb) * 8],
                            num_idxs=nb * 128, num_idxs_reg=nb * 128,
                            elem_size=HROW,
                            queue_num=qbase + qctr[qcls][0] % 2,
                        )
                        qctr[qcls][0] += 1
                        tile.add_dep_helper(gi.ins, sent.ins,
                                            reason="gather waits h")
                        out.append((b0, nb, g_sb))
                    return out

                for t in range(NT):
                    r0 = t * DT
                    nr = min(DT, NPC - r0)
                    if t in pre:
                        s_sb, ix_sb = pre[t]
                    else:
                        s_sb = spool.tile([128, slots], mm1_dt, tag="s")
                        nc.sync.dma_start(s_sb[:], S_d[t])
                        ix_sb = ipool.tile([128, slots // 16], I16, tag="ix")
                        nc.sync.dma_start(ix_sb[:], ix_d[t])
                    chunks = (
                        gather_group(ix_sb, 0, b1, sents["a1"], "a", 0)
                        + gather_group(ix_sb, b1, b1 + b2, sents["a2"], "a", 0)
                        + gather_group(ix_sb, b1 + b2, bmax, sents["b"], "b", 2)
                    )

                    po = popool.tile([128, EMBED], F32)
                    for b0, nb, g_sb in chunks:
                        for bi in range(nb):
                            blk = b0 + bi
                            nc.tensor.matmul(
                                po[:],
                                s_sb[:, blk * 128:(blk + 1) * 128],
                                g_sb[:, bi, :EMBED],
                                start=(blk == 0), stop=(blk == bmax - 1),
                            )
                    o_sm = opool.tile([128, EMBED], F32, tag="osm")
                    nc.vector.tensor_add(o_sm[:], po[:], b_sb[:])
                    if out_dt == F32:
                        o_cast = o_sm
                        nc.vector.tensor_relu(o_cast[:], o_sm[:])
                    else:
                        o_cast = opool.tile([128, EMBED], out_dt, tag="ocast")
                        nc.vector.tensor_relu(o_cast[:], o_sm[:])
                    nc.scalar.dma_start(out_d[r0:r0 + nr, :], o_cast[:nr, :])

    nc.compile()
    return nc


def _preprocess(x, edge_index, edge_weight, W, b, variant):
    cfg = VARIANTS[variant]
    src = np.concatenate([edge_index[0].astype(np.int64),
                          np.arange(N_NODES, dtype=np.int64)])
    dst = np.concatenate([edge_index[1].astype(np.int64),
                          np.arange(N_NODES, dtype=np.int64)])
    wf = np.concatenate([edge_weight.astype(np.float32),
                         np.ones(N_NODES, np.float32)])

    deg = np.bincount(dst, weights=wf.astype(np.float64),
                      minlength=N_NODES).astype(np.float32)
    dis = np.where(deg > 0, 1.0 / np.sqrt(deg), 0.0).astype(np.float32)
    norm = (dis[src] * wf * dis[dst]).astype(np.float32)

    order = np.argsort(dst, kind="stable")
    src_s, dst_s, norm_s = src[order], dst[order], norm[order]

    core_of = dst_s // NPC
    tloc_of = (dst_s % NPC) // DT
    group = core_of * NT + tloc_of
    cnt = np.bincount(group, minlength=N_CORES * NT)
    gstart = np.zeros(N_CORES * NT + 1, np.int64)
    gstart[1:] = np.cumsum(cnt)
    dloc = (dst_s % NPC) % DT

    # Deduplicate sources within each (core, dst-tile): one gather slot per
    # distinct src.  Slots are grouped [a1 | a2 | b] by source row range
    # relative to the core's parity: a1/a2 = quarters of the half this core
    # computes itself, b = the partner's half.
    uniq = []  # (core, tile, u, inv, cuts)
    gmax = [0, 0, 0]
    for g in range(N_CORES * NT):
        lo, hi = gstart[g], gstart[g + 1]
        core = g // NT
        p = core & 1
        u, inv = np.unique(src_s[lo:hi], return_inverse=True)
        my_lo = p * HALF
        # group boundaries in sorted-u index space
        c_a1 = (np.searchsorted(u, my_lo), np.searchsorted(u, my_lo + QUART))
        c_a2 = (c_a1[1], np.searchsorted(u, my_lo + HALF))
        ot_lo = (1 - p) * HALF
        c_b = (np.searchsorted(u, ot_lo), np.searchsorted(u, ot_lo + HALF))
        cuts = (c_a1, c_a2, c_b)
        uniq.append((core, g % NT, u, inv, cuts, lo, hi))
        for i, (cl, ch) in enumerate(cuts):
            gmax[i] = max(gmax[i], ch - cl)
    bmaxes = tuple((m + 127) // 128 for m in gmax)
    gslots = [bm * 128 for bm in bmaxes]
    gbase = [0, gslots[0], gslots[0] + gslots[1]]
    slots = sum(gslots)

    idx_arr = np.zeros((N_CORES, NT, slots), np.int16)
    S_f32 = np.zeros((N_CORES, NT, 128, slots), np.float32)
    for k, t, u, inv, cuts, lo, hi in uniq:
        slot_of = np.empty(len(u), np.int64)
        for gi, (cl, ch) in enumerate(cuts):
            slot_of[cl:ch] = gbase[gi] + np.arange(ch - cl)
            idx_arr[k, t, gbase[gi]:gbase[gi] + ch - cl] = u[cl:ch].astype(np.int16)
        slot = slot_of[inv]
        np.add.at(S_f32[k, t], (slot % 128, (slot // 128) * 128 + dloc[lo:hi]),
                  norm_s[lo:hi])
    S_arr = S_f32.astype(_np_dt(cfg["mm1_dt"]))

    # SWDGE index layout: idx i lives at (partition i%16, col i//16),
    # replicated across the 8 sixteen-partition groups.
    cols = np.arange(slots // 16)
    idx_w = np.empty((N_CORES, NT, 128, slots // 16), np.int16)
    for p in range(16):
        lane = idx_arr[:, :, cols * 16 + p]
        idx_w[:, :, p::16, :] = lane[:, :, None, :]

    x_np = _np_dt(cfg["x_dt"])
    xT = np.zeros((GP, SP), x_np)
    xT[:N_GENES, :N_NODES] = np.ascontiguousarray(
        x.astype(np.float32).T).astype(x_np)
    Wp = np.zeros((GP, EMBED), x_np)
    Wp[:N_GENES] = W.astype(np.float32).astype(x_np)
    bB = np.broadcast_to(b.astype(np.float32), (128, EMBED)).copy()
    return xT, Wp, bB, S_arr, idx_w, bmaxes


def make_in_maps(x, edge_index, edge_weight, W, b, variant=None):
    variant = variant or VARIANT
    xT, Wp, bB, S_arr, idx_w, bmaxes = _preprocess(
        x, edge_index, edge_weight, W, b, variant)
    in_maps = [
        {"xT": np.ascontiguousarray(xT[:, (k & 1) * HALF:((k & 1) + 1) * HALF]),
         "Wp": Wp, "bB": bB, "Sblk": S_arr[k], "idxw": idx_w[k]}
        for k in range(N_CORES)
    ]
    return in_maps, bmaxes


def get_program(bmaxes, variant=None):
    variant = variant or VARIANT
    key = (bmaxes, variant)
    if key not in _prog_cache:
        _prog_cache[key] = _build_program(bmaxes, variant)
    return _prog_cache[key]


def kernel(x, edge_index, edge_weight, W, b):
    x = np.asarray(x)
    edge_index = np.asarray(edge_index)
    edge_weight = np.asarray(edge_weight)
    W = np.asarray(W)
    b = np.asarray(b)

    in_maps, bmaxes = make_in_maps(x, edge_index, edge_weight, W, b)
    nc = get_program(bmaxes)
    res = run_bass_kernel_spmd(nc, in_maps, core_ids=list(range(N_CORES)))
    out = np.concatenate([res.results[k]["out"] for k in range(N_CORES)], axis=0)
    out = np.asarray(out, dtype=np.float32)
    # broadcast-expand (N, E) -> (N, HEADS, E, REP) on host during unshard
    full = np.broadcast_to(out[:, None, :, None],
                           (N_NODES, HEADS, EMBED, REP))
    return np.ascontiguousarray(full)
